# revision 2
# baseline (speedup 1.0000x reference)
"""Trainium2 Bass kernel for the GRU decoder problem (two-stage projection).

Math (reference):
    emb[b,t]   = W_emb @ message[b,t] + b_emb                  # [B,T,E]
    xs[t]      = init_emb (t=0) else emb[:, t-1]               # GRU inputs
    gi[t]      = W_ih @ xs[t] + b_ih                           # [B,3H]
    gh         = W_hh @ h + b_hh
    r          = sigmoid(gi_r + gh_r); z = sigmoid(gi_z + gh_z)
    n          = tanh(gi_n + r * gh_n)
    h'         = (1-z)*n + z*h = n + z*(h - n)
    out        = sigmoid(W_fc2 @ elu(W_fc1 @ h_T + b_fc1) + b_fc2)

Device strategy (pure data parallel over batch, 8 cores, B/core = 512):
  - Two-stage input projection instead of the fused K=512 W_ih@W_emb
    product: stage 1 computes the rank-32 embedding from the fp8 message
    (2 DoubleRow matmuls per step over the full 512 batch), stage 2
    projects emb -> gates with K=33 matmuls whose 33rd contraction row is
    a constant 1 that injects the per-gate bias directly into PSUM.
    This cuts input-side PE work ~7x vs the fused form.
  - Bias-in-PSUM lets one ACTIVATE apply sigmoid to the whole [r|z]
    bank per chain (no per-half bias operand needed).
  - z-form update h' = n + z*(h-n); elementwise ops are spread across
    Vector, Scalar and GpSimd engines (assignment table FLEX below).
  - Step 0 of the GRU is batch-independent (h0 = 0, x0 = init_emb), so h1
    is computed on the host and broadcast; the device scan runs steps
    1..63. Batch is split into two 256-wide chains so one chain's serial
    recurrence path hides under the other's engine work.
"""

import numpy as np

import concourse.bass as bass
import concourse.tile as tile
import concourse.mybir as mybir
from concourse.bass_utils import run_bass_kernel_spmd

N_CORES = 8
B, T, V, E, H, FC, O = 4096, 64, 512, 32, 128, 256, 1024
BS = B // N_CORES      # batch per core
TS = T - 1             # message slices consumed by the GRU
NCH = 2                # batch chains per core
CW = BS // NCH         # chain width
KE = E + 1             # stage-2 contraction: 32 emb features + ones row
WPAD = 48              # stage-1 weight free-dim pad (DR needs step%16==0)

SM = 32.0              # host scale on message (fp8 range centering)
SW1 = 64.0             # host scale on W_emb
S1 = SM * SW1          # scale of stage-1 PSUM output

F8 = mybir.dt.float8e4
F16 = mybir.dt.float16
BF16 = mybir.dt.bfloat16
F32 = mybir.dt.float32
AF = mybir.ActivationFunctionType
OP = mybir.AluOpType
DR = mybir.MatmulPerfMode.DoubleRow

_PROGRAM = None
LAST_RESULTS = None


# walrus codegen in this toolchain encodes at most 1 sem wait per
# instruction; excess waits are hoisted onto NoOp carriers.
_WAIT_LIMITS: dict = {}
_DEFAULT_WAIT_LIMIT = 1


def _split_excess_waits(nc):
    """Hoist sem waits beyond an instruction encoding's capacity onto
    preceding same-engine NoOp carriers (engines execute their queues in
    order, so waiting earlier on the same engine is equivalent)."""
    for f in nc.m.functions:
        for bb in f.blocks:
            newlist = []
            changed = False
            for inst in bb.instructions:
                si = inst.sync_info
                limit = _WAIT_LIMITS.get(type(inst).__name__, _DEFAULT_WAIT_LIMIT)
                if (
                    limit is not None
                    and si is not None
                    and si.on_wait is not None
                    and len(si.on_wait) > limit
                ):
                    waits = list(si.on_wait)
                    for k, w in enumerate(waits[:-limit]):
                        carrier = mybir.InstNoOp(
                            name=f"{inst.name}-wsplit{k}", ins=[], outs=[]
                        )
                        carrier.engine = inst.engine
                        carrier.sync_info = mybir.SyncInfo(on_wait=[w], on_update=[])
                        newlist.append(carrier)
                    si.on_wait = waits[-limit:]
                    inst.sync_info = si
                    changed = True
                newlist.append(inst)
            if changed:
                bb.instructions[:] = newlist


def _build_program():
    nc = bass.Bass()

    # msg[t, p, k, b] = SM * message[b, t, 128k + p]  (fp8 DoubleRow layout)
    msg = nc.dram_tensor("msg", [TS, 128, 4, BS], F8, kind="ExternalInput")
    # w1[p, k, j] = SW1 * W_emb[j, 128k + p] for j<32; cols 32..47 zero.
    w1d = nc.dram_tensor("w1", [128, 4, WPAD], F8, kind="ExternalInput")
    # w2[e, g, m] = W_ih[128g + m, e] for e<32; row 32 = per-gate bias.
    w2d = nc.dram_tensor("w2", [KE, 3, H], BF16, kind="ExternalInput")
    whhT = nc.dram_tensor("whhT", [H, 3 * H], BF16, kind="ExternalInput")
    wfc1T = nc.dram_tensor("wfc1T", [H, FC], BF16, kind="ExternalInput")
    wfc2T = nc.dram_tensor("wfc2T", [FC, O], BF16, kind="ExternalInput")
    # bias columns: 0 b_hn, 1 h1, 2 unused, 3 evict ones col,
    #               4..5 fc1, 6..13 fc2
    biasd = nc.dram_tensor("bias", [128, 14], F32, kind="ExternalInput")
    out = nc.dram_tensor("out", [O // 128, 128, BS], F32, kind="ExternalOutput")

    # elementwise engine assignment for (d, e, h') per chain
    FLEX = [
        (nc.gpsimd, nc.gpsimd, nc.gpsimd),
        (nc.gpsimd, nc.vector, nc.vector),
    ]

    with tile.TileContext(nc) as tc:
        with (
            tc.tile_pool(name="const", bufs=1) as const,
            tc.tile_pool(name="msgp", bufs=8) as msgp,
            tc.tile_pool(name="embp", bufs=2) as embp,
            tc.tile_pool(name="gate", bufs=3) as gate,
            tc.tile_pool(name="fcp", bufs=2) as fcp,
            tc.tile_pool(name="hp", bufs=3) as hp,
            tc.tile_pool(name="outp", bufs=2) as outp,
        ):
            # ---- resident constants ----
            w1_sb = const.tile([128, 4, WPAD], F8)
            nc.sync.dma_start(w1_sb[:], w1d[:])
            w2_sb = const.tile([KE, 3, H], BF16)
            nc.sync.dma_start(w2_sb[:], w2d[:])
            whh_sb = const.tile([H, 3 * H], BF16)
            nc.sync.dma_start(whh_sb[:], whhT[:])
            # fc weights ride the (otherwise idle) gpsimd DMA queue so the
            # big wfc2 transfer doesn't delay the message prologue.
            wfc1_sb = const.tile([H, FC], BF16)
            nc.gpsimd.dma_start(wfc1_sb[:], wfc1T[:])
            wfc2_sb = const.tile([128, FC // 128, O], BF16)
            nc.gpsimd.dma_start(wfc2_sb[:],
                                wfc2T.rearrange("(c p) o -> p c o", p=128))
            bias_sb = const.tile([128, 14], F32)
            nc.sync.dma_start(bias_sb[:], biasd[:])
            zeros = const.tile([128, BS], BF16)
            nc.gpsimd.memset(zeros[:], 0.0)

            # h after step 0 is batch-independent (host-computed) -> broadcast.
            h_tiles = {}
            h0c = []
            for c in range(NCH):
                hc = hp.tile([H, CW], BF16, tag="h" + str(c))
                nc.vector.tensor_scalar_add(hc[:], zeros[:, 0:CW],
                                            bias_sb[:, 1:2])
                h0c.append(hc)
            h_tiles[0] = h0c

            # ---- message DMA prologue ----
            LEAD = 6
            msg_tiles = {}

            def dma_msg(j):
                mt = msgp.tile([128, 4, BS], F8)
                nc.sync.dma_start(mt[:], msg[j])
                msg_tiles[j] = mt

            for j in range(min(LEAD, TS)):
                dma_msg(j)

            emb_tiles = {}
            pa_tiles = {}
            pb_tiles = {}

            with (
                tc.tile_pool(name="psE", bufs=2, space="PSUM") as psE,
                tc.tile_pool(name="psA", bufs=2, space="PSUM") as psA,
                tc.tile_pool(name="psB", bufs=1, space="PSUM") as psB,
            ):

                def emit_stage1(st):
                    # emb(st) for the full batch from msg slice st-1.
                    mt = msg_tiles[st - 1]
                    pe = psE.tile([KE, BS], F32, tag="pE")
                    for kk in (0, 2):
                        nc.tensor.matmul(
                            pe[:],
                            w1_sb[:, kk:kk + 2, 0:KE],
                            mt[:, kk:kk + 2, :],
                            start=(kk == 0),
                            stop=(kk == 2),
                            perf_mode=DR)
                    # evict: embS = pE/S1, ones row from the bias col
                    # (pE row 32 is 0 because w1 col 32 is 0).
                    es = embp.tile([KE, BS], BF16)
                    if st % 2 == 0:
                        nc.vector.tensor_scalar(
                            es[:], pe[:], 1.0 / S1, bias_sb[0:KE, 3:4],
                            op0=OP.mult, op1=OP.add)
                    else:
                        nc.scalar.activation(
                            es[:], pe[:], AF.Identity,
                            bias=bias_sb[0:KE, 3:4], scale=1.0 / S1)
                    emb_tiles[st] = es

                def emit_gi(st):
                    # gate input projections for step st (K=33, bias row
                    # included): r|z into pa, n into pb cols 0:CW.
                    es = emb_tiles[st]
                    pas, pbs = [], []
                    for c in range(NCH):
                        sl = bass.ts(c, CW)
                        pa = psA.tile([128, 2 * CW], F32, tag="pa" + str(c))
                        pb = psB.tile([128, 2 * CW], F32, tag="pb" + str(c))
                        nc.tensor.matmul(pa[:, 0:CW], w2_sb[:, 0, :],
                                         es[:, sl], start=True, stop=False)
                        nc.tensor.matmul(pa[:, CW:2 * CW], w2_sb[:, 1, :],
                                         es[:, sl], start=True, stop=False)
                        nc.tensor.matmul(pb[:, 0:CW], w2_sb[:, 2, :],
                                         es[:, sl], start=True, stop=True)
                        pas.append(pa)
                        pbs.append(pb)
                    pa_tiles[st] = pas
                    pb_tiles[st] = pbs

                emit_stage1(1)
                emit_gi(1)

                for st in range(1, T):
                    # stage-1 matmuls for st+1 go to the PE queue first:
                    # independent work while waiting on h(st-1).
                    if st + 1 <= T - 1:
                        emit_stage1(st + 1)

                    hnew_c = []
                    for c in range(NCH):
                        pa = pa_tiles[st][c]
                        pb = pb_tiles[st][c]
                        hprev = h_tiles[st - 1][c]
                        eng_d, eng_e, eng_h = FLEX[c]

                        # gh matmuls accumulate onto the gi pre-activations
                        nc.tensor.matmul(pa[:, 0:CW], whh_sb[:, 0:H],
                                         hprev[:], start=False, stop=True)
                        nc.tensor.matmul(pa[:, CW:2 * CW], whh_sb[:, H:2 * H],
                                         hprev[:], start=False, stop=True)
                        nc.tensor.matmul(pb[:, CW:2 * CW],
                                         whh_sb[:, 2 * H:3 * H],
                                         hprev[:], start=True, stop=True)

                        # r|z in one ACTIVATE (biases already in PSUM)
                        rz = gate.tile([128, 2 * CW], BF16, tag="rz" + str(c))
                        nc.scalar.activation(rz[:], pa[:], AF.Sigmoid)
                        # rh = (gh_n + b_hn) * r
                        rh = gate.tile([128, CW], F16, tag="rh" + str(c))
                        nc.vector.scalar_tensor_tensor(
                            rh[:], pb[:, CW:2 * CW], bias_sb[:, 0:1],
                            rz[:, 0:CW], op0=OP.add, op1=OP.mult)
                        # s = gi_n + rh   (b_in already in PSUM)
                        s = gate.tile([128, CW], F16, tag="s" + str(c))
                        nc.vector.scalar_tensor_tensor(
                            s[:], pb[:, 0:CW], 0.0, rh[:],
                            op0=OP.add, op1=OP.add)
                        nng = gate.tile([128, CW], BF16, tag="n" + str(c))
                        nc.scalar.activation(nng[:], s[:], AF.Tanh)
                        # h' = n + z*(h - n)
                        d = gate.tile([128, CW], BF16, tag="d" + str(c))
                        eng_d.tensor_tensor(d[:], hprev[:], nng[:],
                                            op=OP.subtract)
                        e = gate.tile([128, CW], BF16, tag="e" + str(c))
                        eng_e.tensor_tensor(e[:], rz[:, CW:2 * CW], d[:],
                                            op=OP.mult)
                        hnew = hp.tile([H, CW], BF16, tag="h" + str(c))
                        eng_h.tensor_tensor(hnew[:], nng[:], e[:], op=OP.add)
                        hnew_c.append(hnew)
                    h_tiles[st] = hnew_c

                    if st + 1 <= T - 1:
                        emit_gi(st + 1)

                    j = LEAD + st - 1
                    if j < TS:
                        dma_msg(j)

            # ---- output head: fc1 + ELU, fc2 + sigmoid ----
            with tc.tile_pool(name="psf", bufs=2, space="PSUM") as psf:
                hlast = h_tiles[T - 1]
                hid = []
                for c in range(FC // 128):
                    pf = psf.tile([128, BS], F32, tag="pf")
                    for ch in range(NCH):
                        nc.tensor.matmul(pf[:, bass.ts(ch, CW)],
                                         wfc1_sb[:, bass.ts(c, 128)],
                                         hlast[ch][:],
                                         start=True, stop=True)
                    bcol = bias_sb[:, 4 + c:5 + c]
                    x1 = fcp.tile([128, BS], BF16, tag="fcx")
                    nc.vector.tensor_scalar_add(x1[:], pf[:], bcol)
                    e1 = fcp.tile([128, BS], F32, tag="fce")
                    nc.scalar.activation(e1[:], pf[:], AF.Exp, bias=bcol)
                    # elu(x) = max(x,0) + min(exp(x)-1, 0)
                    em = fcp.tile([128, BS], BF16, tag="fcm")
                    nc.vector.scalar_tensor_tensor(em[:], e1[:], -1.0,
                                                   zeros[:],
                                                   op0=OP.add, op1=OP.min)
                    hc = fcp.tile([128, BS], BF16, tag="hid" + str(c))
                    nc.vector.scalar_tensor_tensor(hc[:], x1[:], 0.0, em[:],
                                                   op0=OP.max, op1=OP.add)
                    hid.append(hc)
                for o in range(O // 128):
                    po = psf.tile([128, BS], F32, tag="po")
                    for c in range(FC // 128):
                        nc.tensor.matmul(po[:], wfc2_sb[:, c, bass.ts(o, 128)],
                                         hid[c][:], start=(c == 0),
                                         stop=(c == FC // 128 - 1))
                    ob = outp.tile([128, BS], F32)
                    nc.scalar.activation(ob[:], po[:], AF.Sigmoid,
                                         bias=bias_sb[:, 6 + o:7 + o])
                    nc.sync.dma_start(out[o], ob[:])

    _split_excess_waits(nc)
    return nc


def _sigmoid(x):
    return 1.0 / (1.0 + np.exp(-x))


def kernel(message, W_emb, b_emb, init_emb, W_ih, W_hh, b_ih, b_hh,
           W_fc1, b_fc1, W_fc2, b_fc2, _trace=False, _trace_kwargs=None):
    global _PROGRAM, LAST_RESULTS
    if _PROGRAM is None:
        _PROGRAM = _build_program()
    nc = _PROGRAM

    f32 = np.float32

    import ml_dtypes
    bf16 = ml_dtypes.bfloat16
    fp8 = ml_dtypes.float8_e4m3fn

    # message -> per-core fp8 DoubleRow layout [t, p, k, b]; the last token's
    # embedding is never consumed by the GRU so only t = 0..62 is shipped.
    msgT = (
        (message[:, :TS, :] * SM)
        .reshape(N_CORES, BS, TS, 4, 128)
        .transpose(0, 2, 4, 3, 1)  # [core, t, p, k, b]
        .astype(fp8)
    )
    msgT = np.ascontiguousarray(msgT)

    # stage-1 weights: W_emb^T in fp8 DoubleRow layout, padded to 48 cols
    w1 = np.zeros((128, 4, WPAD), f32)
    w1[:, :, :E] = (SW1 * W_emb.astype(np.float64)).T.reshape(
        4, 128, E).transpose(1, 0, 2)
    w1 = w1.astype(fp8)

    # stage-2 weights: K=33 with bias row
    b_combo = (W_ih.astype(np.float64) @ b_emb.astype(np.float64)
               + b_ih.astype(np.float64))
    b_hh64 = b_hh.astype(np.float64)
    w2 = np.zeros((KE, 3, H), np.float64)
    w2[:E] = W_ih.astype(np.float64).reshape(3, H, E).transpose(2, 0, 1)
    w2[E, 0] = (b_combo + b_hh64)[0:H]
    w2[E, 1] = (b_combo + b_hh64)[H:2 * H]
    w2[E, 2] = b_combo[2 * H:3 * H]
    w2 = w2.astype(bf16)

    whhT_ = np.ascontiguousarray(W_hh.astype(np.float64).T).astype(bf16)
    wfc1T = np.ascontiguousarray(W_fc1.T).astype(bf16)
    wfc2T = np.ascontiguousarray(W_fc2.T).astype(bf16)

    # step 0 is batch-independent: h0 = 0, x0 = init_emb
    gi0 = W_ih.astype(np.float64) @ init_emb[0, 0].astype(np.float64) + b_ih
    r0 = _sigmoid(gi0[0:H] + b_hh64[0:H])
    z0 = _sigmoid(gi0[H:2 * H] + b_hh64[H:2 * H])
    n0 = np.tanh(gi0[2 * H:3 * H] + r0 * b_hh64[2 * H:3 * H])
    h1 = (1.0 - z0) * n0

    bias = np.zeros((128, 14), f32)
    bias[:, 0] = b_hh64[2 * H:3 * H]
    bias[:, 1] = h1
    bias[KE - 1, 3] = 1.0
    bias[:, 4:6] = b_fc1.reshape(2, 128).T.astype(f32)
    bias[:, 6:14] = b_fc2.reshape(8, 128).T.astype(f32)

    shared = dict(w1=w1, w2=w2, whhT=whhT_, wfc1T=wfc1T,
                  wfc2T=wfc2T, bias=bias)
    in_maps = [dict(msg=msgT[c], **shared) for c in range(N_CORES)]

    kw = dict(_trace_kwargs or {})
    res = run_bass_kernel_spmd(nc, in_maps, list(range(N_CORES)),
                               trace=_trace, **kw)
    LAST_RESULTS = res

    outs = [res.results[c]["out"].reshape(O, BS).T for c in range(N_CORES)]
    return np.ascontiguousarray(np.concatenate(outs, axis=0), dtype=f32)


# revision 5
# speedup vs baseline: 1.1433x; 1.1433x over previous
"""Trainium2 Bass kernel for the GRU decoder problem (two-stage projection).

Math (reference):
    emb[b,t]   = W_emb @ message[b,t] + b_emb                  # [B,T,E]
    xs[t]      = init_emb (t=0) else emb[:, t-1]               # GRU inputs
    gi[t]      = W_ih @ xs[t] + b_ih                           # [B,3H]
    gh         = W_hh @ h + b_hh
    r          = sigmoid(gi_r + gh_r); z = sigmoid(gi_z + gh_z)
    n          = tanh(gi_n + r * gh_n)
    h'         = (1-z)*n + z*h = n + z*(h - n)
    out        = sigmoid(W_fc2 @ elu(W_fc1 @ h_T + b_fc1) + b_fc2)

Device strategy (pure data parallel over batch, 8 cores, B/core = 512):
  - Two-stage input projection instead of the fused K=512 W_ih@W_emb
    product: stage 1 computes the rank-32 embedding from the fp8 message
    (2 DoubleRow matmuls per step over the full 512 batch), stage 2
    projects emb -> gates with K=33 matmuls whose 33rd contraction row is
    a constant 1 that injects the per-gate bias directly into PSUM.
    This cuts input-side PE work ~7x vs the fused form.
  - Bias-in-PSUM lets one ACTIVATE apply sigmoid to the whole [r|z]
    bank per chain (no per-half bias operand needed).
  - z-form update h' = n + z*(h-n); elementwise ops are spread across
    Vector, Scalar and GpSimd engines (assignment table FLEX below).
  - Step 0 of the GRU is batch-independent (h0 = 0, x0 = init_emb), so h1
    is computed on the host and broadcast; the device scan runs steps
    1..63. Batch is split into two 256-wide chains so one chain's serial
    recurrence path hides under the other's engine work.
"""

import numpy as np

import concourse.bass as bass
import concourse.tile as tile
import concourse.mybir as mybir
from concourse.bass_utils import run_bass_kernel_spmd

N_CORES = 8
B, T, V, E, H, FC, O = 4096, 64, 512, 32, 128, 256, 1024
BS = B // N_CORES      # batch per core
TS = T - 1             # message slices consumed by the GRU
NCH = 2                # batch chains per core
CW = BS // NCH         # chain width
KE = E + 1             # stage-2 contraction: 32 emb features + ones row
WPAD = 48              # stage-1 weight free-dim pad (DR needs step%16==0)

SM = 32.0              # host scale on message (fp8 range centering)
SW1 = 64.0             # host scale on W_emb
S1 = SM * SW1          # scale of stage-1 PSUM output

F8 = mybir.dt.float8e4
F16 = mybir.dt.float16
BF16 = mybir.dt.bfloat16
F32 = mybir.dt.float32
AF = mybir.ActivationFunctionType
OP = mybir.AluOpType
DR = mybir.MatmulPerfMode.DoubleRow

_PROGRAM = None
LAST_RESULTS = None


# walrus codegen in this toolchain encodes at most 1 sem wait per
# instruction; excess waits are hoisted onto NoOp carriers.
_WAIT_LIMITS: dict = {}
_DEFAULT_WAIT_LIMIT = 1


def _split_excess_waits(nc):
    """Hoist sem waits beyond an instruction encoding's capacity onto
    preceding same-engine NoOp carriers (engines execute their queues in
    order, so waiting earlier on the same engine is equivalent)."""
    for f in nc.m.functions:
        for bb in f.blocks:
            newlist = []
            changed = False
            for inst in bb.instructions:
                si = inst.sync_info
                limit = _WAIT_LIMITS.get(type(inst).__name__, _DEFAULT_WAIT_LIMIT)
                if (
                    limit is not None
                    and si is not None
                    and si.on_wait is not None
                    and len(si.on_wait) > limit
                ):
                    waits = list(si.on_wait)
                    for k, w in enumerate(waits[:-limit]):
                        carrier = mybir.InstNoOp(
                            name=f"{inst.name}-wsplit{k}", ins=[], outs=[]
                        )
                        carrier.engine = inst.engine
                        carrier.sync_info = mybir.SyncInfo(on_wait=[w], on_update=[])
                        newlist.append(carrier)
                    si.on_wait = waits[-limit:]
                    inst.sync_info = si
                    changed = True
                newlist.append(inst)
            if changed:
                bb.instructions[:] = newlist


def _build_program():
    nc = bass.Bass()

    # msg[t, p, k, b] = SM * message[b, t, 128k + p]  (fp8 DoubleRow layout)
    msg = nc.dram_tensor("msg", [TS, 128, 4, BS], F8, kind="ExternalInput")
    # w1[p, k, j] = SW1 * W_emb[j, 128k + p] for j<32; cols 32..47 zero.
    w1d = nc.dram_tensor("w1", [128, 4, WPAD], F8, kind="ExternalInput")
    # w2[e, g, m] = W_ih[128g + m, e] for e<32; row 32 = per-gate bias.
    w2d = nc.dram_tensor("w2", [KE, 3, H], BF16, kind="ExternalInput")
    whhT = nc.dram_tensor("whhT", [H, 3 * H], BF16, kind="ExternalInput")
    wfc1T = nc.dram_tensor("wfc1T", [H, FC], BF16, kind="ExternalInput")
    wfc2T = nc.dram_tensor("wfc2T", [FC, O], BF16, kind="ExternalInput")
    # bias columns: 0 b_hn, 1 h1, 2 unused, 3 evict ones col,
    #               4..5 fc1, 6..13 fc2
    biasd = nc.dram_tensor("bias", [128, 14], F32, kind="ExternalInput")
    out = nc.dram_tensor("out", [O // 128, 128, BS], F32, kind="ExternalOutput")

    # elementwise engine assignment for (d, e, h') per chain
    FLEX = [
        (nc.vector, nc.vector, nc.vector),
        (nc.vector, nc.vector, nc.vector),
    ]

    with tile.TileContext(nc) as tc:
        with (
            tc.tile_pool(name="const", bufs=1) as const,
            tc.tile_pool(name="msgp", bufs=8) as msgp,
            tc.tile_pool(name="embp", bufs=2) as embp,
            tc.tile_pool(name="gate", bufs=3) as gate,
            tc.tile_pool(name="fcp", bufs=2) as fcp,
            tc.tile_pool(name="hp", bufs=3) as hp,
            tc.tile_pool(name="outp", bufs=2) as outp,
        ):
            # ---- resident constants ----
            w1_sb = const.tile([128, 4, WPAD], F8)
            nc.sync.dma_start(w1_sb[:], w1d[:])
            w2_sb = const.tile([KE, 3, H], BF16)
            nc.sync.dma_start(w2_sb[:], w2d[:])
            whh_sb = const.tile([H, 3 * H], BF16)
            nc.sync.dma_start(whh_sb[:], whhT[:])
            # fc weights ride the (otherwise idle) gpsimd DMA queue so the
            # big wfc2 transfer doesn't delay the message prologue.
            wfc1_sb = const.tile([H, FC], BF16)
            nc.gpsimd.dma_start(wfc1_sb[:], wfc1T[:])
            wfc2_sb = const.tile([128, FC // 128, O], BF16)
            nc.gpsimd.dma_start(wfc2_sb[:],
                                wfc2T.rearrange("(c p) o -> p c o", p=128))
            bias_sb = const.tile([128, 14], F32)
            nc.sync.dma_start(bias_sb[:], biasd[:])
            zeros = const.tile([128, BS], BF16)
            nc.gpsimd.memset(zeros[:], 0.0)

            # h after step 0 is batch-independent (host-computed) -> broadcast.
            h_tiles = {}
            h0c = []
            for c in range(NCH):
                hc = hp.tile([H, CW], BF16, tag="h" + str(c))
                nc.vector.tensor_scalar_add(hc[:], zeros[:, 0:CW],
                                            bias_sb[:, 1:2])
                h0c.append(hc)
            h_tiles[0] = h0c

            # ---- message DMA prologue ----
            LEAD = 6
            msg_tiles = {}

            def dma_msg(j):
                mt = msgp.tile([128, 4, BS], F8)
                nc.sync.dma_start(mt[:], msg[j])
                msg_tiles[j] = mt

            for j in range(min(LEAD, TS)):
                dma_msg(j)

            emb_tiles = {}
            pa_tiles = {}
            pb_tiles = {}

            with (
                tc.tile_pool(name="psE", bufs=2, space="PSUM") as psE,
                tc.tile_pool(name="psA", bufs=2, space="PSUM") as psA,
                tc.tile_pool(name="psB", bufs=1, space="PSUM") as psB,
            ):

                def emit_stage1(st):
                    # emb(st) for the full batch from msg slice st-1.
                    mt = msg_tiles[st - 1]
                    pe = psE.tile([KE, BS], F32, tag="pE")
                    for kk in (0, 2):
                        nc.tensor.matmul(
                            pe[:],
                            w1_sb[:, kk:kk + 2, 0:KE],
                            mt[:, kk:kk + 2, :],
                            start=(kk == 0),
                            stop=(kk == 2),
                            perf_mode=DR)
                    # evict: embS = pE/S1, ones row from the bias col
                    # (pE row 32 is 0 because w1 col 32 is 0). ACT keeps
                    # this off the DVE, which carries the recurrence tail.
                    es = embp.tile([KE, BS], BF16)
                    nc.scalar.activation(
                        es[:], pe[:], AF.Identity,
                        bias=bias_sb[0:KE, 3:4], scale=1.0 / S1)
                    emb_tiles[st] = es

                def emit_gi(st):
                    # gate input projections for step st (K=33, bias row
                    # included): r|z into pa, n into pb cols 0:CW.
                    es = emb_tiles[st]
                    pas, pbs = [], []
                    for c in range(NCH):
                        sl = bass.ts(c, CW)
                        pa = psA.tile([128, 2 * CW], F32, tag="pa" + str(c))
                        pb = psB.tile([128, 2 * CW], F32, tag="pb" + str(c))
                        nc.tensor.matmul(pa[:, 0:CW], w2_sb[:, 0, :],
                                         es[:, sl], start=True, stop=False)
                        nc.tensor.matmul(pa[:, CW:2 * CW], w2_sb[:, 1, :],
                                         es[:, sl], start=True, stop=False)
                        nc.tensor.matmul(pb[:, 0:CW], w2_sb[:, 2, :],
                                         es[:, sl], start=True, stop=True)
                        pas.append(pa)
                        pbs.append(pb)
                    pa_tiles[st] = pas
                    pb_tiles[st] = pbs

                emit_stage1(1)
                emit_gi(1)

                for st in range(1, T):
                    # stage-1 matmuls for st+1 go to the PE queue first:
                    # independent work while waiting on h(st-1).
                    if st + 1 <= T - 1:
                        emit_stage1(st + 1)

                    # Stage-major emission: every engine's in-order queue
                    # sees chain 0's op directly followed by chain 1's, so
                    # the chains pipeline instead of serializing behind
                    # each other's later stages.
                    for c in range(NCH):
                        pa = pa_tiles[st][c]
                        pb = pb_tiles[st][c]
                        hprev = h_tiles[st - 1][c]
                        # gh matmuls accumulate onto the gi pre-activations
                        nc.tensor.matmul(pa[:, 0:CW], whh_sb[:, 0:H],
                                         hprev[:], start=False, stop=True)
                        nc.tensor.matmul(pa[:, CW:2 * CW], whh_sb[:, H:2 * H],
                                         hprev[:], start=False, stop=True)
                        nc.tensor.matmul(pb[:, CW:2 * CW],
                                         whh_sb[:, 2 * H:3 * H],
                                         hprev[:], start=True, stop=True)

                    rz_c, rh_c, s_c, nng_c, d_c, e_c = {}, {}, {}, {}, {}, {}
                    for c in range(NCH):
                        # r|z in one ACTIVATE (biases already in PSUM)
                        rz = gate.tile([128, 2 * CW], BF16, tag="rz" + str(c))
                        nc.scalar.activation(rz[:], pa_tiles[st][c][:],
                                             AF.Sigmoid)
                        rz_c[c] = rz
                    for c in range(NCH):
                        pb = pb_tiles[st][c]
                        # rh = (gh_n + b_hn) * r
                        rh = gate.tile([128, CW], F16, tag="rh" + str(c))
                        nc.vector.scalar_tensor_tensor(
                            rh[:], pb[:, CW:2 * CW], bias_sb[:, 0:1],
                            rz_c[c][:, 0:CW], op0=OP.add, op1=OP.mult)
                        # s = gi_n + rh   (b_in already in PSUM)
                        s = gate.tile([128, CW], F16, tag="s" + str(c))
                        nc.vector.scalar_tensor_tensor(
                            s[:], pb[:, 0:CW], 0.0, rh[:],
                            op0=OP.add, op1=OP.add)
                        rh_c[c], s_c[c] = rh, s
                    for c in range(NCH):
                        nng = gate.tile([128, CW], BF16, tag="n" + str(c))
                        nc.scalar.activation(nng[:], s_c[c][:], AF.Tanh)
                        nng_c[c] = nng
                    for c in range(NCH):
                        # h' = n + z*(h - n)
                        eng_d, eng_e, eng_h = FLEX[c]
                        d = gate.tile([128, CW], BF16, tag="d" + str(c))
                        eng_d.tensor_tensor(d[:], h_tiles[st - 1][c][:],
                                            nng_c[c][:], op=OP.subtract)
                        d_c[c] = d
                    for c in range(NCH):
                        eng_d, eng_e, eng_h = FLEX[c]
                        e = gate.tile([128, CW], BF16, tag="e" + str(c))
                        eng_e.tensor_tensor(e[:], rz_c[c][:, CW:2 * CW],
                                            d_c[c][:], op=OP.mult)
                        e_c[c] = e
                    hnew_c = []
                    for c in range(NCH):
                        eng_d, eng_e, eng_h = FLEX[c]
                        hnew = hp.tile([H, CW], BF16, tag="h" + str(c))
                        eng_h.tensor_tensor(hnew[:], nng_c[c][:], e_c[c][:],
                                            op=OP.add)
                        hnew_c.append(hnew)
                    h_tiles[st] = hnew_c

                    if st + 1 <= T - 1:
                        emit_gi(st + 1)

                    j = LEAD + st - 1
                    if j < TS:
                        dma_msg(j)

            # ---- output head: fc1 + ELU, fc2 + sigmoid ----
            with tc.tile_pool(name="psf", bufs=2, space="PSUM") as psf:
                hlast = h_tiles[T - 1]
                hid = []
                for c in range(FC // 128):
                    pf = psf.tile([128, BS], F32, tag="pf")
                    for ch in range(NCH):
                        nc.tensor.matmul(pf[:, bass.ts(ch, CW)],
                                         wfc1_sb[:, bass.ts(c, 128)],
                                         hlast[ch][:],
                                         start=True, stop=True)
                    bcol = bias_sb[:, 4 + c:5 + c]
                    x1 = fcp.tile([128, BS], BF16, tag="fcx")
                    nc.vector.tensor_scalar_add(x1[:], pf[:], bcol)
                    e1 = fcp.tile([128, BS], F32, tag="fce")
                    nc.scalar.activation(e1[:], pf[:], AF.Exp, bias=bcol)
                    # elu(x) = max(x,0) + min(exp(x)-1, 0)
                    em = fcp.tile([128, BS], BF16, tag="fcm")
                    nc.vector.scalar_tensor_tensor(em[:], e1[:], -1.0,
                                                   zeros[:],
                                                   op0=OP.add, op1=OP.min)
                    hc = fcp.tile([128, BS], BF16, tag="hid" + str(c))
                    nc.vector.scalar_tensor_tensor(hc[:], x1[:], 0.0, em[:],
                                                   op0=OP.max, op1=OP.add)
                    hid.append(hc)
                for o in range(O // 128):
                    po = psf.tile([128, BS], F32, tag="po")
                    for c in range(FC // 128):
                        nc.tensor.matmul(po[:], wfc2_sb[:, c, bass.ts(o, 128)],
                                         hid[c][:], start=(c == 0),
                                         stop=(c == FC // 128 - 1))
                    ob = outp.tile([128, BS], F32)
                    nc.scalar.activation(ob[:], po[:], AF.Sigmoid,
                                         bias=bias_sb[:, 6 + o:7 + o])
                    nc.sync.dma_start(out[o], ob[:])

    _split_excess_waits(nc)
    return nc


def _sigmoid(x):
    return 1.0 / (1.0 + np.exp(-x))


def kernel(message, W_emb, b_emb, init_emb, W_ih, W_hh, b_ih, b_hh,
           W_fc1, b_fc1, W_fc2, b_fc2, _trace=False, _trace_kwargs=None):
    global _PROGRAM, LAST_RESULTS
    if _PROGRAM is None:
        _PROGRAM = _build_program()
    nc = _PROGRAM

    f32 = np.float32

    import ml_dtypes
    bf16 = ml_dtypes.bfloat16
    fp8 = ml_dtypes.float8_e4m3fn

    # message -> per-core fp8 DoubleRow layout [t, p, k, b]; the last token's
    # embedding is never consumed by the GRU so only t = 0..62 is shipped.
    msgT = (
        (message[:, :TS, :] * SM)
        .reshape(N_CORES, BS, TS, 4, 128)
        .transpose(0, 2, 4, 3, 1)  # [core, t, p, k, b]
        .astype(fp8)
    )
    msgT = np.ascontiguousarray(msgT)

    # stage-1 weights: W_emb^T in fp8 DoubleRow layout, padded to 48 cols
    w1 = np.zeros((128, 4, WPAD), f32)
    w1[:, :, :E] = (SW1 * W_emb.astype(np.float64)).T.reshape(
        4, 128, E).transpose(1, 0, 2)
    w1 = w1.astype(fp8)

    # stage-2 weights: K=33 with bias row
    b_combo = (W_ih.astype(np.float64) @ b_emb.astype(np.float64)
               + b_ih.astype(np.float64))
    b_hh64 = b_hh.astype(np.float64)
    w2 = np.zeros((KE, 3, H), np.float64)
    w2[:E] = W_ih.astype(np.float64).reshape(3, H, E).transpose(2, 0, 1)
    w2[E, 0] = (b_combo + b_hh64)[0:H]
    w2[E, 1] = (b_combo + b_hh64)[H:2 * H]
    w2[E, 2] = b_combo[2 * H:3 * H]
    w2 = w2.astype(bf16)

    whhT_ = np.ascontiguousarray(W_hh.astype(np.float64).T).astype(bf16)
    wfc1T = np.ascontiguousarray(W_fc1.T).astype(bf16)
    wfc2T = np.ascontiguousarray(W_fc2.T).astype(bf16)

    # step 0 is batch-independent: h0 = 0, x0 = init_emb
    gi0 = W_ih.astype(np.float64) @ init_emb[0, 0].astype(np.float64) + b_ih
    r0 = _sigmoid(gi0[0:H] + b_hh64[0:H])
    z0 = _sigmoid(gi0[H:2 * H] + b_hh64[H:2 * H])
    n0 = np.tanh(gi0[2 * H:3 * H] + r0 * b_hh64[2 * H:3 * H])
    h1 = (1.0 - z0) * n0

    bias = np.zeros((128, 14), f32)
    bias[:, 0] = b_hh64[2 * H:3 * H]
    bias[:, 1] = h1
    bias[KE - 1, 3] = 1.0
    bias[:, 4:6] = b_fc1.reshape(2, 128).T.astype(f32)
    bias[:, 6:14] = b_fc2.reshape(8, 128).T.astype(f32)

    shared = dict(w1=w1, w2=w2, whhT=whhT_, wfc1T=wfc1T,
                  wfc2T=wfc2T, bias=bias)
    in_maps = [dict(msg=msgT[c], **shared) for c in range(N_CORES)]

    kw = dict(_trace_kwargs or {})
    res = run_bass_kernel_spmd(nc, in_maps, list(range(N_CORES)),
                               trace=_trace, **kw)
    LAST_RESULTS = res

    outs = [res.results[c]["out"].reshape(O, BS).T for c in range(N_CORES)]
    return np.ascontiguousarray(np.concatenate(outs, axis=0), dtype=f32)


# revision 6
# speedup vs baseline: 1.4145x; 1.2372x over previous
"""Trainium2 Bass kernel for the GRU decoder problem (two-stage projection).

Math (reference):
    emb[b,t]   = W_emb @ message[b,t] + b_emb                  # [B,T,E]
    xs[t]      = init_emb (t=0) else emb[:, t-1]               # GRU inputs
    gi[t]      = W_ih @ xs[t] + b_ih                           # [B,3H]
    gh         = W_hh @ h + b_hh
    r          = sigmoid(gi_r + gh_r); z = sigmoid(gi_z + gh_z)
    n          = tanh(gi_n + r * gh_n)
    h'         = (1-z)*n + z*h = n + z*(h - n)
    out        = sigmoid(W_fc2 @ elu(W_fc1 @ h_T + b_fc1) + b_fc2)

Device strategy (pure data parallel over batch, 8 cores, B/core = 512):
  - Two-stage input projection instead of the fused K=512 W_ih@W_emb
    product: stage 1 computes the rank-32 embedding from the fp8 message
    (2 DoubleRow matmuls per step over the full 512 batch), stage 2
    projects emb -> gates with K=33 matmuls whose 33rd contraction row is
    a constant 1 that injects the per-gate bias directly into PSUM.
    This cuts input-side PE work ~7x vs the fused form.
  - Bias-in-PSUM lets one ACTIVATE apply sigmoid to the whole [r|z]
    bank per chain (no per-half bias operand needed).
  - z-form update h' = n + z*(h-n); elementwise ops are spread across
    Vector, Scalar and GpSimd engines (assignment table FLEX below).
  - Step 0 of the GRU is batch-independent (h0 = 0, x0 = init_emb), so h1
    is computed on the host and broadcast; the device scan runs steps
    1..63. Batch is split into two 256-wide chains so one chain's serial
    recurrence path hides under the other's engine work.
"""

import numpy as np

import concourse.bass as bass
import concourse.tile as tile
import concourse.mybir as mybir
from concourse.bass_utils import run_bass_kernel_spmd

N_CORES = 8
B, T, V, E, H, FC, O = 4096, 64, 512, 32, 128, 256, 1024
BS = B // N_CORES      # batch per core
TS = T - 1             # message slices consumed by the GRU
NCH = 2                # batch chains per core
CW = BS // NCH         # chain width
KE = E + 1             # stage-2 contraction: 32 emb features + ones row
WPAD = 48              # stage-1 weight free-dim pad (DR needs step%16==0)

SM = 32.0              # host scale on message (fp8 range centering)
SW1 = 64.0             # host scale on W_emb
S1 = SM * SW1          # scale of stage-1 PSUM output

F8 = mybir.dt.float8e4
F16 = mybir.dt.float16
BF16 = mybir.dt.bfloat16
F32 = mybir.dt.float32
AF = mybir.ActivationFunctionType
OP = mybir.AluOpType
DR = mybir.MatmulPerfMode.DoubleRow

_PROGRAM = None
LAST_RESULTS = None


# walrus codegen in this toolchain encodes at most 1 sem wait per
# instruction; excess waits are hoisted onto NoOp carriers.
_WAIT_LIMITS: dict = {}
_DEFAULT_WAIT_LIMIT = 1


def _split_excess_waits(nc):
    """Hoist sem waits beyond an instruction encoding's capacity onto
    preceding same-engine NoOp carriers (engines execute their queues in
    order, so waiting earlier on the same engine is equivalent)."""
    for f in nc.m.functions:
        for bb in f.blocks:
            newlist = []
            changed = False
            for inst in bb.instructions:
                si = inst.sync_info
                limit = _WAIT_LIMITS.get(type(inst).__name__, _DEFAULT_WAIT_LIMIT)
                if (
                    limit is not None
                    and si is not None
                    and si.on_wait is not None
                    and len(si.on_wait) > limit
                ):
                    waits = list(si.on_wait)
                    for k, w in enumerate(waits[:-limit]):
                        carrier = mybir.InstNoOp(
                            name=f"{inst.name}-wsplit{k}", ins=[], outs=[]
                        )
                        carrier.engine = inst.engine
                        carrier.sync_info = mybir.SyncInfo(on_wait=[w], on_update=[])
                        newlist.append(carrier)
                    si.on_wait = waits[-limit:]
                    inst.sync_info = si
                    changed = True
                newlist.append(inst)
            if changed:
                bb.instructions[:] = newlist


def _build_program():
    nc = bass.Bass()

    # msg[t, p, k, b] = SM * message[b, t, 128k + p]  (fp8 DoubleRow layout)
    msg = nc.dram_tensor("msg", [TS, 128, 4, BS], F8, kind="ExternalInput")
    # w1[p, k, j] = SW1 * W_emb[j, 128k + p] for j<32; cols 32..47 zero.
    w1d = nc.dram_tensor("w1", [128, 4, WPAD], F8, kind="ExternalInput")
    # w2[e, g, m] = W_ih[128g + m, e] for e<32; row 32 = per-gate bias.
    w2d = nc.dram_tensor("w2", [KE, 3, H], BF16, kind="ExternalInput")
    whhT = nc.dram_tensor("whhT", [H, 3 * H], BF16, kind="ExternalInput")
    wfc1T = nc.dram_tensor("wfc1T", [H, FC], BF16, kind="ExternalInput")
    wfc2T = nc.dram_tensor("wfc2T", [FC, O], BF16, kind="ExternalInput")
    # bias columns: 0 b_hn, 1 h1, 2 unused, 3 evict ones col,
    #               4..5 fc1, 6..13 fc2
    biasd = nc.dram_tensor("bias", [128, 14], F32, kind="ExternalInput")
    out = nc.dram_tensor("out", [O // 128, 128, BS], F32, kind="ExternalOutput")

    # elementwise engine assignment for (d, e, h') per chain
    FLEX = [
        (nc.vector, nc.vector, nc.vector),
        (nc.vector, nc.vector, nc.vector),
    ]

    with tile.TileContext(nc) as tc:
        with (
            tc.tile_pool(name="const", bufs=1) as const,
            tc.tile_pool(name="msgp", bufs=8) as msgp,
            tc.tile_pool(name="embp", bufs=2) as embp,
            tc.tile_pool(name="gate", bufs=3) as gate,
            tc.tile_pool(name="fcp", bufs=2) as fcp,
            tc.tile_pool(name="hp", bufs=3) as hp,
            tc.tile_pool(name="outp", bufs=2) as outp,
        ):
            # ---- resident constants ----
            w1_sb = const.tile([128, 4, WPAD], F8)
            nc.sync.dma_start(w1_sb[:], w1d[:])
            w2_sb = const.tile([KE, 3, H], BF16)
            nc.sync.dma_start(w2_sb[:], w2d[:])
            whh_sb = const.tile([H, 3 * H], BF16)
            nc.sync.dma_start(whh_sb[:], whhT[:])
            # fc weights ride the (otherwise idle) gpsimd DMA queue so the
            # big wfc2 transfer doesn't delay the message prologue.
            wfc1_sb = const.tile([H, FC], BF16)
            nc.gpsimd.dma_start(wfc1_sb[:], wfc1T[:])
            wfc2_sb = const.tile([128, FC // 128, O], BF16)
            nc.gpsimd.dma_start(wfc2_sb[:],
                                wfc2T.rearrange("(c p) o -> p c o", p=128))
            bias_sb = const.tile([128, 14], F32)
            nc.sync.dma_start(bias_sb[:], biasd[:])
            zeros = const.tile([128, BS], BF16)
            nc.gpsimd.memset(zeros[:], 0.0)

            # h after step 0 is batch-independent (host-computed) -> broadcast.
            h_tiles = {}
            h0c = []
            for c in range(NCH):
                hc = hp.tile([H, CW], BF16, tag="h" + str(c))
                nc.vector.tensor_scalar_add(hc[:], zeros[:, 0:CW],
                                            bias_sb[:, 1:2])
                h0c.append(hc)
            h_tiles[0] = h0c

            # ---- message DMA prologue ----
            LEAD = 6
            msg_tiles = {}

            def dma_msg(j):
                mt = msgp.tile([128, 4, BS], F8)
                nc.sync.dma_start(mt[:], msg[j])
                msg_tiles[j] = mt

            for j in range(min(LEAD, TS)):
                dma_msg(j)

            emb_tiles = {}
            pa_tiles = {}
            pb_tiles = {}

            with (
                tc.tile_pool(name="psE", bufs=2, space="PSUM") as psE,
                tc.tile_pool(name="psA", bufs=2, space="PSUM") as psA,
                tc.tile_pool(name="psB", bufs=1, space="PSUM") as psB,
            ):

                def emit_stage1(st):
                    # emb(st) for the full batch from msg slice st-1.
                    mt = msg_tiles[st - 1]
                    pe = psE.tile([KE, BS], F32, tag="pE")
                    for kk in (0, 2):
                        nc.tensor.matmul(
                            pe[:],
                            w1_sb[:, kk:kk + 2, 0:KE],
                            mt[:, kk:kk + 2, :],
                            start=(kk == 0),
                            stop=(kk == 2),
                            perf_mode=DR)
                    # evict: embS = pE/S1, ones row from the bias col
                    # (pE row 32 is 0 because w1 col 32 is 0). ACT keeps
                    # this off the DVE, which carries the recurrence tail.
                    es = embp.tile([KE, BS], BF16)
                    nc.scalar.activation(
                        es[:], pe[:], AF.Identity,
                        bias=bias_sb[0:KE, 3:4], scale=1.0 / S1)
                    emb_tiles[st] = es

                def emit_gi(st):
                    # gate input projections for step st (K=33, bias row
                    # included): r|z into pa, n into pb cols 0:CW.
                    es = emb_tiles[st]
                    pas, pbs = [], []
                    for c in range(NCH):
                        sl = bass.ts(c, CW)
                        pa = psA.tile([128, 2 * CW], F32, tag="pa" + str(c))
                        pb = psB.tile([128, 2 * CW], F32, tag="pb" + str(c))
                        nc.tensor.matmul(pa[:, 0:CW], w2_sb[:, 0, :],
                                         es[:, sl], start=True, stop=False)
                        nc.tensor.matmul(pa[:, CW:2 * CW], w2_sb[:, 1, :],
                                         es[:, sl], start=True, stop=False)
                        nc.tensor.matmul(pb[:, 0:CW], w2_sb[:, 2, :],
                                         es[:, sl], start=True, stop=True)
                        pas.append(pa)
                        pbs.append(pb)
                    pa_tiles[st] = pas
                    pb_tiles[st] = pbs

                emit_stage1(1)
                emit_gi(1)

                # uv-split state: gh(t) = Whh@nng(t-1) + Whh@e(t-1), so the
                # materialized h' (GpSimd, off-path) only feeds d(t+1) and
                # the output head.
                nng_prev = None
                e_prev = None

                for st in range(1, T):
                    # stage-1 matmuls for st+1 go to the PE queue first:
                    # independent work while waiting on the recurrence.
                    if st + 1 <= T - 1:
                        emit_stage1(st + 1)

                    # gh matmuls accumulate onto the gi pre-activations.
                    # r-gate first (sigma_r is the path head), n-gate last.
                    def gh_mms(region_of, g):
                        for c in range(NCH):
                            reg = region_of(c)
                            w = whh_sb[:, g * H:(g + 1) * H]
                            st_flag = (g == 2)
                            if st == 1:
                                nc.tensor.matmul(reg, w, h_tiles[0][c][:],
                                                 start=st_flag, stop=True)
                            else:
                                nc.tensor.matmul(reg, w, nng_prev[c][:],
                                                 start=st_flag, stop=False)
                                nc.tensor.matmul(reg, w, e_prev[c][:],
                                                 start=False, stop=True)

                    gh_mms(lambda c: pa_tiles[st][c][:, 0:CW], 0)
                    gh_mms(lambda c: pa_tiles[st][c][:, CW:2 * CW], 1)
                    gh_mms(lambda c: pb_tiles[st][c][:, CW:2 * CW], 2)

                    r_c, z_c, rh_c, s_c, nng_c, d_c, e_c = ({} for _ in
                                                            range(7))
                    for c in range(NCH):
                        r = gate.tile([128, CW], BF16, tag="r" + str(c))
                        nc.scalar.activation(r[:], pa_tiles[st][c][:, 0:CW],
                                             AF.Sigmoid)
                        r_c[c] = r
                    for c in range(NCH):
                        z = gate.tile([128, CW], BF16, tag="z" + str(c))
                        nc.scalar.activation(z[:],
                                             pa_tiles[st][c][:, CW:2 * CW],
                                             AF.Sigmoid)
                        z_c[c] = z
                    for c in range(NCH):
                        pb = pb_tiles[st][c]
                        # rh = (gh_n + b_hn) * r
                        rh = gate.tile([128, CW], F16, tag="rh" + str(c))
                        nc.vector.scalar_tensor_tensor(
                            rh[:], pb[:, CW:2 * CW], bias_sb[:, 0:1],
                            r_c[c][:], op0=OP.add, op1=OP.mult)
                        # s = gi_n + rh   (b_in already in PSUM)
                        s = gate.tile([128, CW], F16, tag="s" + str(c))
                        nc.vector.scalar_tensor_tensor(
                            s[:], pb[:, 0:CW], 0.0, rh[:],
                            op0=OP.add, op1=OP.add)
                        rh_c[c], s_c[c] = rh, s
                    for c in range(NCH):
                        nng = gate.tile([128, CW], BF16, tag="n" + str(c))
                        nc.scalar.activation(nng[:], s_c[c][:], AF.Tanh)
                        nng_c[c] = nng
                    for c in range(NCH):
                        # e = z*(h - n); h' = n + e (off-path, GpSimd)
                        d = gate.tile([128, CW], BF16, tag="d" + str(c))
                        nc.vector.tensor_tensor(d[:], h_tiles[st - 1][c][:],
                                                nng_c[c][:], op=OP.subtract)
                        d_c[c] = d
                    for c in range(NCH):
                        e = gate.tile([128, CW], BF16, tag="e" + str(c))
                        nc.vector.tensor_tensor(e[:], z_c[c][:], d_c[c][:],
                                                op=OP.mult)
                        e_c[c] = e
                    hnew_c = []
                    for c in range(NCH):
                        hnew = hp.tile([H, CW], BF16, tag="h" + str(c))
                        nc.gpsimd.tensor_tensor(hnew[:], nng_c[c][:],
                                                e_c[c][:], op=OP.add)
                        hnew_c.append(hnew)
                    h_tiles[st] = hnew_c
                    nng_prev = nng_c
                    e_prev = e_c

                    if st + 1 <= T - 1:
                        emit_gi(st + 1)

                    j = LEAD + st - 1
                    if j < TS:
                        dma_msg(j)

            # ---- output head: fc1 + ELU, fc2 + sigmoid ----
            with tc.tile_pool(name="psf", bufs=2, space="PSUM") as psf:
                hlast = h_tiles[T - 1]
                hid = []
                for c in range(FC // 128):
                    pf = psf.tile([128, BS], F32, tag="pf")
                    for ch in range(NCH):
                        nc.tensor.matmul(pf[:, bass.ts(ch, CW)],
                                         wfc1_sb[:, bass.ts(c, 128)],
                                         hlast[ch][:],
                                         start=True, stop=True)
                    bcol = bias_sb[:, 4 + c:5 + c]
                    x1 = fcp.tile([128, BS], BF16, tag="fcx")
                    nc.vector.tensor_scalar_add(x1[:], pf[:], bcol)
                    e1 = fcp.tile([128, BS], F32, tag="fce")
                    nc.scalar.activation(e1[:], pf[:], AF.Exp, bias=bcol)
                    # elu(x) = max(x,0) + min(exp(x)-1, 0)
                    em = fcp.tile([128, BS], BF16, tag="fcm")
                    nc.vector.scalar_tensor_tensor(em[:], e1[:], -1.0,
                                                   zeros[:],
                                                   op0=OP.add, op1=OP.min)
                    hc = fcp.tile([128, BS], BF16, tag="hid" + str(c))
                    nc.vector.scalar_tensor_tensor(hc[:], x1[:], 0.0, em[:],
                                                   op0=OP.max, op1=OP.add)
                    hid.append(hc)
                for o in range(O // 128):
                    po = psf.tile([128, BS], F32, tag="po")
                    for c in range(FC // 128):
                        nc.tensor.matmul(po[:], wfc2_sb[:, c, bass.ts(o, 128)],
                                         hid[c][:], start=(c == 0),
                                         stop=(c == FC // 128 - 1))
                    ob = outp.tile([128, BS], F32)
                    nc.scalar.activation(ob[:], po[:], AF.Sigmoid,
                                         bias=bias_sb[:, 6 + o:7 + o])
                    nc.sync.dma_start(out[o], ob[:])

    _split_excess_waits(nc)
    return nc


def _sigmoid(x):
    return 1.0 / (1.0 + np.exp(-x))


def kernel(message, W_emb, b_emb, init_emb, W_ih, W_hh, b_ih, b_hh,
           W_fc1, b_fc1, W_fc2, b_fc2, _trace=False, _trace_kwargs=None):
    global _PROGRAM, LAST_RESULTS
    if _PROGRAM is None:
        _PROGRAM = _build_program()
    nc = _PROGRAM

    f32 = np.float32

    import ml_dtypes
    bf16 = ml_dtypes.bfloat16
    fp8 = ml_dtypes.float8_e4m3fn

    # message -> per-core fp8 DoubleRow layout [t, p, k, b]; the last token's
    # embedding is never consumed by the GRU so only t = 0..62 is shipped.
    msgT = (
        (message[:, :TS, :] * SM)
        .reshape(N_CORES, BS, TS, 4, 128)
        .transpose(0, 2, 4, 3, 1)  # [core, t, p, k, b]
        .astype(fp8)
    )
    msgT = np.ascontiguousarray(msgT)

    # stage-1 weights: W_emb^T in fp8 DoubleRow layout, padded to 48 cols
    w1 = np.zeros((128, 4, WPAD), f32)
    w1[:, :, :E] = (SW1 * W_emb.astype(np.float64)).T.reshape(
        4, 128, E).transpose(1, 0, 2)
    w1 = w1.astype(fp8)

    # stage-2 weights: K=33 with bias row
    b_combo = (W_ih.astype(np.float64) @ b_emb.astype(np.float64)
               + b_ih.astype(np.float64))
    b_hh64 = b_hh.astype(np.float64)
    w2 = np.zeros((KE, 3, H), np.float64)
    w2[:E] = W_ih.astype(np.float64).reshape(3, H, E).transpose(2, 0, 1)
    w2[E, 0] = (b_combo + b_hh64)[0:H]
    w2[E, 1] = (b_combo + b_hh64)[H:2 * H]
    w2[E, 2] = b_combo[2 * H:3 * H]
    w2 = w2.astype(bf16)

    whhT_ = np.ascontiguousarray(W_hh.astype(np.float64).T).astype(bf16)
    wfc1T = np.ascontiguousarray(W_fc1.T).astype(bf16)
    wfc2T = np.ascontiguousarray(W_fc2.T).astype(bf16)

    # step 0 is batch-independent: h0 = 0, x0 = init_emb
    gi0 = W_ih.astype(np.float64) @ init_emb[0, 0].astype(np.float64) + b_ih
    r0 = _sigmoid(gi0[0:H] + b_hh64[0:H])
    z0 = _sigmoid(gi0[H:2 * H] + b_hh64[H:2 * H])
    n0 = np.tanh(gi0[2 * H:3 * H] + r0 * b_hh64[2 * H:3 * H])
    h1 = (1.0 - z0) * n0

    bias = np.zeros((128, 14), f32)
    bias[:, 0] = b_hh64[2 * H:3 * H]
    bias[:, 1] = h1
    bias[KE - 1, 3] = 1.0
    bias[:, 4:6] = b_fc1.reshape(2, 128).T.astype(f32)
    bias[:, 6:14] = b_fc2.reshape(8, 128).T.astype(f32)

    shared = dict(w1=w1, w2=w2, whhT=whhT_, wfc1T=wfc1T,
                  wfc2T=wfc2T, bias=bias)
    in_maps = [dict(msg=msgT[c], **shared) for c in range(N_CORES)]

    kw = dict(_trace_kwargs or {})
    res = run_bass_kernel_spmd(nc, in_maps, list(range(N_CORES)),
                               trace=_trace, **kw)
    LAST_RESULTS = res

    outs = [res.results[c]["out"].reshape(O, BS).T for c in range(N_CORES)]
    return np.ascontiguousarray(np.concatenate(outs, axis=0), dtype=f32)
